# revision 3
# baseline (speedup 1.0000x reference)
"""Trainium2 Bass kernel for nn_Circuit (batch quantum circuit simulation).

Math: the circuit is u1 on every qubit, u2 on disjoint adjacent pairs, u1 on
every qubit again.  Since all gates factor over the 10 disjoint qubit pairs,
the whole circuit equals M^{tensor 10} with M = (u1 x u1) @ u2 @ (u1 x u1),
a single 4x4 complex matrix applied to every base-4 digit of the state index.

Strategy (data-parallel, one batch item per core):
  State as real fp32 [128, 16384]; partition bits = (c | 3 digits) where c is
  the re/im component bit.  Three matmul passes with realified 128x128
  stationaries contract digits 3..6 (super-pass with d6 blocked in the free
  dim), 7..9, and 0..2; two full 128x128 PE-transpose exchanges move digit
  groups between partitions and free dim; c alternates partition/free and the
  middle pass mixes re/im via PSUM accumulation over component slices.
  All matmuls/transposes in fp32r (TF32); PSUM accumulation fp32.
"""
import numpy as np

import concourse.bacc as bacc
import concourse.tile as tile
import concourse.mybir as mybir
from concourse.bass_utils import run_bass_kernel_spmd

F32 = mybir.dt.float32
F32R = mybir.dt.float32r

NQ = 20
BATCH = 8
DIM = 2 ** NQ
N_CORES = 8

_NC_CACHE = {}


def _realify(W):
    """lhsT [128,128] for out[(c',w)] = sum_{c,v} L[(c,v),(c',w)] x[(c,v)],
    complex W acting as out[w] = sum_v W[w,v] x[v]."""
    Wr, Wi = W.real, W.imag
    L = np.zeros((128, 128), np.float64)
    L[0:64, 0:64] = Wr.T
    L[64:128, 0:64] = -Wi.T
    L[0:64, 64:128] = Wi.T
    L[64:128, 64:128] = Wr.T
    return L


def build_weights(u1_re, u1_im, u2_re, u2_im):
    """Host-side: fused gate M, Kronecker powers, realified stationaries.
    Returns [128, 22*128] fp32 (22 matrices packed side by side)."""
    u1 = np.asarray(u1_re, np.float64) + 1j * np.asarray(u1_im, np.float64)
    u2 = np.asarray(u2_re, np.float64) + 1j * np.asarray(u2_im, np.float64)
    A = np.kron(u1, u1)
    M = A @ u2 @ A                      # 4x4 complex
    W3 = np.kron(M, np.kron(M, M))      # 64x64 complex, digit-major

    mats = []
    # 0..15: pass A stationaries, blocks (j = d6 in, i = e6 out), m = j*4+i
    for j in range(4):
        for i in range(4):
            mats.append(_realify(M[i, j] * W3))
    # 16..19: pass B stationaries SB[c][c'] = kron(I2, C_cc'.T), m = 16+c*2+c'
    Wr, Wi = W3.real, W3.imag
    C = {(0, 0): Wr, (0, 1): Wi, (1, 0): -Wi, (1, 1): Wr}
    for c in range(2):
        for cp in range(2):
            mats.append(np.kron(np.eye(2), C[(c, cp)].T))
    # 20: pass C stationary
    mats.append(_realify(W3))
    # 21: identity for PE transposes
    mats.append(np.eye(128))

    wts = np.stack(mats)                                  # [22,128,128]
    packed = wts.transpose(1, 0, 2).reshape(128, 22 * 128)
    return np.ascontiguousarray(packed).astype(np.float32)


def build_nc():
    nc = bacc.Bacc("TRN2", target_bir_lowering=False, debug=False,
                   num_devices=N_CORES)
    xin_d = nc.dram_tensor("xin", [128, 16384], F32R, kind="ExternalInput").ap()
    wts_d = nc.dram_tensor("wts", [128, 22 * 128], F32R,
                           kind="ExternalInput").ap()
    xout_d = nc.dram_tensor("xout", [128, 16384], F32,
                            kind="ExternalOutput").ap()

    dcnt = [0]

    with tile.TileContext(nc) as tc:
        with tc.tile_pool(name="sb", bufs=1) as sb, \
             tc.tile_pool(name="ps", bufs=2, space="PSUM") as ps:

            wt = sb.tile([128, 22 * 128], F32R, tag="wt")
            nc.sync.dma_start(wt[:], wts_d)

            def W(m):
                return wt[:, m * 128:(m + 1) * 128]

            b1 = [sb.tile([128, 2048], F32R, tag=f"b1_{k}", name=f"b1_{k}")
                  for k in range(8)]
            b2 = [sb.tile([128, 2048], F32R, tag=f"b2_{k}", name=f"b2_{k}")
                  for k in range(8)]
            bC = sb.tile([128, 16384], F32R, tag="bC")

            for k in range(8):
                nc.sync.dma_start(b1[k][:], xin_d[:, 2048 * k:2048 * (k + 1)])

            def drain(out_ap, in_ap):
                if dcnt[0] % 2 == 0:
                    nc.scalar.copy(out_ap, in_ap)
                else:
                    nc.vector.tensor_copy(out_ap, in_ap)
                dcnt[0] += 1

            # ---- pass A: contract d3,d4,d5 (partitions) + d6 (free blocks)
            # b1 layout L_A: f = v012*256 + d6*64 + r789
            # out -> b2 layout L_B: f = v012*256 + e6*64 + r789
            for ck in range(8):
                pt = ps.tile([128, 2048], F32, tag="ps")
                rhsv = b1[ck][:].rearrange("p (v j r) -> p j v r",
                                           v=8, j=4, r=64)
                for i in range(4):
                    for j in range(4):
                        nc.tensor.matmul(pt[:, i * 512:(i + 1) * 512],
                                         W(j * 4 + i), rhsv[:, j],
                                         start=(j == 0), stop=(j == 3))
                outv = b2[ck][:].rearrange("p (v e r) -> p e v r",
                                           v=8, e=4, r=64)
                inv = pt[:].rearrange("p (e v r) -> p e v r", e=4, v=8, r=64)
                drain(outv, inv)

            # ---- E1: full transposes; in-cols = (e6l, r789) contiguous 128
            # out -> bC layout L_C: f = w345*256 + e6h*128 + c*64 + v012
            for g in range(8):
                pt = ps.tile([128, 2048], F32, tag="ps")
                for v8 in range(8):
                    for e6h in range(2):
                        bidx = v8 * 2 + e6h
                        off = v8 * 256 + e6h * 128
                        nc.tensor.transpose(
                            pt[:, bidx * 128:(bidx + 1) * 128].bitcast(F32R),
                            b2[g][:, off:off + 128], W(21))
                outv = bC[:].rearrange("p (w ec v) -> p v ec w",
                                       w=64, ec=4, v=64)[:, 8 * g:8 * g + 8]
                inv = pt[:].rearrange("p (v ec w) -> p v ec w",
                                      v=8, ec=4, w=64)
                drain(outv, inv)

            # ---- pass B: contract d7,d8,d9 (partitions (e6l, r789)),
            # c in free; accumulate over c slices, c' -> free bit.
            # out -> b1 layout L_D: f = w345*256 + e6h*128 + c'*64 + v012
            bCv = bC[:].rearrange("p (t we c v) -> p c t we v",
                                  t=16, we=8, c=2, v=64)
            for q in range(8):
                pt = ps.tile([128, 2048], F32, tag="ps")
                for cp in range(2):
                    for tl in range(2):
                        tp = 2 * q + tl
                        for c in range(2):
                            nc.tensor.matmul(
                                pt[:, cp * 1024 + tl * 512:
                                   cp * 1024 + (tl + 1) * 512],
                                W(16 + c * 2 + cp), bCv[:, c, tp],
                                start=(c == 0), stop=(c == 1))
                outv = b1[q][:].rearrange("p (twe c v) -> p c twe v",
                                          twe=16, c=2, v=64)
                inv = pt[:].rearrange("p (c twe v) -> p c twe v",
                                      c=2, twe=16, v=64)
                drain(outv, inv)

            # ---- E2: full transposes; in-cols = (c', v012) contiguous 128
            # out -> b2 layout L_E: f = w345*256 + e6*64 + w789 (plain copy)
            for q in range(8):
                pt = ps.tile([128, 2048], F32, tag="ps")
                for w8 in range(8):
                    for e6h in range(2):
                        bidx = w8 * 2 + e6h
                        off = w8 * 256 + e6h * 128
                        nc.tensor.transpose(
                            pt[:, bidx * 128:(bidx + 1) * 128].bitcast(F32R),
                            b1[q][:, off:off + 128], W(21))
                drain(b2[q][:], pt[:])

            # ---- pass C: contract d0,d1,d2 with c' (partitions (c', v012))
            # out partitions (c, e0e1e2); f already e3..e9 natural order.
            for q in range(8):
                pt = ps.tile([128, 2048], F32, tag="ps")
                for u in range(4):
                    nc.tensor.matmul(pt[:, u * 512:(u + 1) * 512], W(20),
                                     b2[q][:, u * 512:(u + 1) * 512],
                                     start=True, stop=True)
                drain(b1[q][:], pt[:])
                nc.sync.dma_start(xout_d[:, 2048 * q:2048 * (q + 1)],
                                  b1[q][:].bitcast(F32))

    nc.compile()
    return nc


def _get_nc():
    if "nc" not in _NC_CACHE:
        _NC_CACHE["nc"] = build_nc()
    return _NC_CACHE["nc"]


def pack_state(x_real, x_imag, b):
    """[DIM] re/im planes of batch item b -> [128, 16384] load layout L_A."""
    arr = np.stack([np.asarray(x_real[b], np.float32),
                    np.asarray(x_imag[b], np.float32)])        # [2, DIM]
    # i = v012*16384 + v345*256 + d6*64 + r789
    arr = arr.reshape(2, 64, 64, 4, 64).transpose(0, 2, 1, 3, 4)
    return np.ascontiguousarray(arr.reshape(128, 16384))


def unpack_state(xout):
    """[128, 16384] final layout -> ([DIM] re, [DIM] im)."""
    return xout[0:64].reshape(-1), xout[64:128].reshape(-1)


def kernel(x_real, x_imag, u1_re, u1_im, u2_re, u2_im):
    nc = _get_nc()
    wts = build_weights(u1_re, u1_im, u2_re, u2_im)
    in_maps = [{"xin": pack_state(x_real, x_imag, b), "wts": wts}
               for b in range(BATCH)]
    res = run_bass_kernel_spmd(nc, in_maps, core_ids=list(range(N_CORES)))
    out = np.empty((2, BATCH, DIM), np.float32)
    for b in range(BATCH):
        re, im = unpack_state(res.results[b]["xout"])
        out[0, b] = re
        out[1, b] = im
    return out
